# revision 16
# baseline (speedup 1.0000x reference)
"""Block-DCT quantizer (8x8 DCT -> quant/dequant -> IDCT) on 8 Trainium2 cores.

Sharding: pure data parallel over batch. Core b processes x[b] = [3, 1024, 1024],
flattened to [3072, 1024], in 48 chunks of [128, 512].

Key trick: bf16 is the top half of fp32, so a stride-2 bf16 access pattern over
an fp32 SBUF tile IS its bf16 truncation — and the PE streams it at full bf16
rate (measured). Intermediates therefore stay fp32, each layout flip is a DVE
32x32-block transpose reading PSUM directly (fused with the evacuation), and
matmuls consume bf16 views for free:

    S1  colDCT      ps1 = Dbig @ bf16view(X)         (PE)
    F1  transpose   y1t = blkT(ps1)     PSUM->SBUF   (DVE, fp32)
        partition becomes (h' div 32, w mod 32); w mod 8 stays 8-aligned, so
        the row DCT in this layout is the same block-diagonal Dbig.
    S2  rowDCT/q    ps2 = (Dbig/qstep) @ bf16view(y1t)
    Q1  round       qi  = int32(ps2)    PSUM->SBUF   (ACT, exact RNE cast)
    Q2  cast        q2  = bf16(qi)      SBUF->SBUF   (DVE 2x mode, exact)
    S3  rowIDCT*q   ps3 = (qstep*Dbig^T) @ q2
    F2  transpose   zt  = blkT(ps3)     PSUM->SBUF   (DVE, involution)
    S4  colIDCT     ps4 = Dbig^T @ bf16view(zt)
    E4  evac        o   = ps4           PSUM->SBUF   (ACT)

Quantized coefficients land on exact integers (zero for sane inputs), so bf16
truncation cannot perturb the rounding decision and the output matches the
fp32 reference exactly.

The loop is emitted software-pipelined (one sub-stage per tick, deepest stage
first) so each engine's in-order queue interleaves chunks instead of executing
the serial per-chunk dependency chain.
"""
import math
import sys

sys.path.insert(0, "/opt/trn_rl_repo")

import ml_dtypes
import numpy as np

import concourse.bass as bass  # noqa: F401
import concourse.mybir as mybir
import concourse.tile as tile
from concourse import bacc, bass_utils

P = 128
CW = 512         # chunk width = matmul free dim = one PSUM bank of fp32
N_CORES = 8

_BUILD_CACHE = {}


def _dct_matrix(n: int) -> np.ndarray:
    k = np.arange(n, dtype=np.float64)[:, None]
    j = np.arange(n, dtype=np.float64)[None, :]
    d = np.cos(math.pi / n * (j + 0.5) * k)
    scale = np.full((n, 1), math.sqrt(2.0 / n))
    scale[0, 0] = math.sqrt(1.0 / n)
    return d * scale


def _bf16_view(ap):
    # top 16 bits of each little-endian fp32 element = its bf16 truncation
    return ap.bitcast(mybir.dt.bfloat16)[:, 1::2]


def _build(rows: int, width: int):
    key = (rows, width)
    if key in _BUILD_CACHE:
        return _BUILD_CACHE[key]

    assert rows % P == 0 and width % CW == 0
    n_strips = rows // P
    n_wchunks = width // CW
    f32 = mybir.dt.float32
    bf16 = mybir.dt.bfloat16
    i32 = mybir.dt.int32

    nc = bacc.Bacc("TRN2", target_bir_lowering=False, debug=False,
                   num_devices=N_CORES)
    x = nc.dram_tensor("x", [rows, width], f32, kind="ExternalInput").ap()
    ms = [
        nc.dram_tensor(f"m{i}", [P, P], bf16, kind="ExternalInput").ap()
        for i in range(1, 5)
    ]
    y = nc.dram_tensor("y", [rows, width], f32, kind="ExternalOutput").ap()

    with tile.TileContext(nc) as tc:
        with tc.tile_pool(name="consts", bufs=1) as cpool, \
             tc.tile_pool(name="io", bufs=8) as iopool, \
             tc.tile_pool(name="mid", bufs=8) as midpool, \
             tc.tile_pool(name="psum", bufs=8, space="PSUM") as psum:
            mt = []
            for i, m in enumerate(ms):
                t = cpool.tile([P, P], bf16, tag=f"m{i}", name=f"mt{i}")
                nc.gpsimd.dma_start(out=t, in_=m)
                mt.append(t)
            m1t, m2t, m3t, m4t = mt

            chunks = [(s * P, c * CW)
                      for s in range(n_strips) for c in range(n_wchunks)]
            st = [dict() for _ in chunks]

            def stage(k, i):
                v = st[i]
                r0, c0 = chunks[i]
                if k == 0:
                    v["xt"] = iopool.tile([P, CW], f32, tag="xt", name="xt")
                    nc.gpsimd.dma_start(out=v["xt"],
                                        in_=x[r0:r0 + P, c0:c0 + CW])
                elif k == 1:
                    v["ps1"] = psum.tile([P, CW], f32, tag="ps", name="ps")
                    nc.tensor.matmul(v["ps1"], lhsT=m1t,
                                     rhs=_bf16_view(v.pop("xt")),
                                     start=True, stop=True)
                elif k == 2:
                    v["y1t"] = midpool.tile([P, CW], f32, tag="y1t",
                                            name="y1t")
                    nc.vector.transpose(out=v["y1t"], in_=v.pop("ps1"))
                elif k == 3:
                    v["ps2"] = psum.tile([P, CW], f32, tag="ps", name="ps")
                    nc.tensor.matmul(v["ps2"], lhsT=m2t,
                                     rhs=_bf16_view(v.pop("y1t")),
                                     start=True, stop=True)
                elif k == 4:
                    v["qi"] = midpool.tile([P, CW], i32, tag="qi", name="qi")
                    nc.scalar.copy(v["qi"], v.pop("ps2"))
                elif k == 5:
                    v["q2"] = midpool.tile([P, CW], bf16, tag="q2", name="q2")
                    nc.vector.tensor_copy(out=v["q2"], in_=v.pop("qi"))
                elif k == 6:
                    v["ps3"] = psum.tile([P, CW], f32, tag="ps", name="ps")
                    nc.tensor.matmul(v["ps3"], lhsT=m3t, rhs=v.pop("q2"),
                                     start=True, stop=True)
                elif k == 7:
                    v["zt"] = midpool.tile([P, CW], f32, tag="zt", name="zt")
                    nc.vector.transpose(out=v["zt"], in_=v.pop("ps3"))
                elif k == 8:
                    v["ps4"] = psum.tile([P, CW], f32, tag="ps", name="ps")
                    nc.tensor.matmul(v["ps4"], lhsT=m4t,
                                     rhs=_bf16_view(v.pop("zt")),
                                     start=True, stop=True)
                elif k == 9:
                    v["o"] = iopool.tile([P, CW], f32, tag="o", name="o")
                    nc.scalar.copy(v["o"], v.pop("ps4"))
                elif k == 10:
                    nc.sync.dma_start(out=y[r0:r0 + P, c0:c0 + CW],
                                      in_=v.pop("o"))

            n_stages = 11
            for t in range(len(chunks) + n_stages - 1):
                for k in range(n_stages - 1, -1, -1):  # deepest stage first
                    i = t - k
                    if 0 <= i < len(chunks):
                        stage(k, i)

    nc.compile()
    _BUILD_CACHE[key] = nc
    return nc


def kernel(x: np.ndarray, block_size, qp, _trace: bool = False,
           _results_out: list | None = None) -> np.ndarray:
    n = int(block_size)
    qp = int(qp)
    b, ch, h, w = x.shape
    assert P % n == 0, f"block size {n} must divide {P}"
    # the 32x32 block-transpose keeps w mod 32 in the partition dim; the row
    # DCT stays block-diagonal iff n divides 32
    assert 32 % n == 0, f"block size {n} must divide 32"
    assert h % n == 0 and w % n == 0, "padding path not implemented"
    assert b == N_CORES, f"expected batch {N_CORES}, got {b}"
    rows = ch * h
    assert rows % P == 0 and w % CW == 0

    qstep = float(np.float32(2.0 ** ((qp - 4.0) / 6.0)))
    d = _dct_matrix(n)
    dbig = np.kron(np.eye(P // n), d)
    m1 = dbig.T            # colDCT:      out = Dbig @ X
    m2 = dbig.T / qstep    # rowDCT/q     (same Dbig in the flipped layout)
    m3 = qstep * dbig      # rowIDCT*q
    m4 = dbig              # colIDCT
    consts = {
        f"m{i}": np.ascontiguousarray(m.astype(ml_dtypes.bfloat16))
        for i, m in enumerate((m1, m2, m3, m4), start=1)
    }

    nc = _build(rows, w)
    x_np = np.asarray(x, dtype=np.float32)
    in_maps = [
        {"x": np.ascontiguousarray(x_np[i].reshape(rows, w)), **consts}
        for i in range(N_CORES)
    ]
    res = bass_utils.run_bass_kernel_spmd(
        nc, in_maps, core_ids=list(range(N_CORES)), trace=_trace)
    if _results_out is not None:
        _results_out.append(res)
    out = np.stack([res.results[i]["y"].reshape(ch, h, w)
                    for i in range(N_CORES)])
    return out


# revision 17
# speedup vs baseline: 1.0150x; 1.0150x over previous
"""Block-DCT quantizer (8x8 DCT -> quant/dequant -> IDCT) on 8 Trainium2 cores.

Sharding: pure data parallel over batch. Core b processes x[b] = [3, 1024, 1024],
flattened to [3072, 1024], in 48 chunks of [128, 512].

Key trick: bf16 is the top half of fp32, so a stride-2 bf16 access pattern over
an fp32 SBUF tile IS its bf16 truncation — and the PE streams it at full bf16
rate (measured). Intermediates therefore stay fp32, each layout flip is a DVE
32x32-block transpose reading PSUM directly (fused with the evacuation), and
matmuls consume bf16 views for free:

    S1  colDCT      ps1 = Dbig @ bf16view(X)         (PE)
    F1  transpose   y1t = blkT(ps1)     PSUM->SBUF   (DVE, fp32)
        partition becomes (h' div 32, w mod 32); w mod 8 stays 8-aligned, so
        the row DCT in this layout is the same block-diagonal Dbig.
    S2  rowDCT/q    ps2 = (Dbig/qstep) @ bf16view(y1t)
    Q1  round       qi  = int32(ps2)    PSUM->SBUF   (ACT, exact RNE cast)
    Q2  cast        q2  = bf16(qi)      SBUF->SBUF   (DVE 2x mode, exact)
    S3  rowIDCT*q   ps3 = (qstep*Dbig^T) @ q2
    F2  transpose   zt  = blkT(ps3)     PSUM->SBUF   (DVE, involution)
    S4  colIDCT     ps4 = Dbig^T @ bf16view(zt)
    E4  evac        o   = ps4           PSUM->SBUF   (ACT)

Quantized coefficients land on exact integers (zero for sane inputs), so bf16
truncation cannot perturb the rounding decision and the output matches the
fp32 reference exactly.

The loop is emitted software-pipelined (one sub-stage per tick, deepest stage
first) so each engine's in-order queue interleaves chunks instead of executing
the serial per-chunk dependency chain.
"""
import math
import sys

sys.path.insert(0, "/opt/trn_rl_repo")

import ml_dtypes
import numpy as np

import concourse.bass as bass  # noqa: F401
import concourse.mybir as mybir
import concourse.tile as tile
from concourse import bacc, bass_utils

P = 128
CW = 512         # chunk width = matmul free dim = one PSUM bank of fp32
N_CORES = 8

_BUILD_CACHE = {}


def _dct_matrix(n: int) -> np.ndarray:
    k = np.arange(n, dtype=np.float64)[:, None]
    j = np.arange(n, dtype=np.float64)[None, :]
    d = np.cos(math.pi / n * (j + 0.5) * k)
    scale = np.full((n, 1), math.sqrt(2.0 / n))
    scale[0, 0] = math.sqrt(1.0 / n)
    return d * scale


def _bf16_view(ap):
    # top 16 bits of each little-endian fp32 element = its bf16 truncation
    return ap.bitcast(mybir.dt.bfloat16)[:, 1::2]


def _build(rows: int, width: int):
    key = (rows, width)
    if key in _BUILD_CACHE:
        return _BUILD_CACHE[key]

    assert rows % P == 0 and width % CW == 0
    n_strips = rows // P
    n_wchunks = width // CW
    f32 = mybir.dt.float32
    bf16 = mybir.dt.bfloat16
    i32 = mybir.dt.int32

    nc = bacc.Bacc("TRN2", target_bir_lowering=False, debug=False,
                   num_devices=N_CORES)
    x = nc.dram_tensor("x", [rows, width], f32, kind="ExternalInput").ap()
    ms = [
        nc.dram_tensor(f"m{i}", [P, P], bf16, kind="ExternalInput").ap()
        for i in range(1, 5)
    ]
    y = nc.dram_tensor("y", [rows, width], f32, kind="ExternalOutput").ap()

    with tile.TileContext(nc) as tc:
        with tc.tile_pool(name="consts", bufs=1) as cpool, \
             tc.tile_pool(name="io", bufs=8) as iopool, \
             tc.tile_pool(name="mid", bufs=8) as midpool, \
             tc.tile_pool(name="psum", bufs=8, space="PSUM") as psum:
            mt = []
            for i, m in enumerate(ms):
                t = cpool.tile([P, P], bf16, tag=f"m{i}", name=f"mt{i}")
                nc.gpsimd.dma_start(out=t, in_=m)
                mt.append(t)
            m1t, m2t, m3t, m4t = mt

            chunks = [(s * P, c * CW)
                      for s in range(n_strips) for c in range(n_wchunks)]
            st = [dict() for _ in chunks]

            def stage(k, i):
                v = st[i]
                r0, c0 = chunks[i]
                if k == 0:
                    v["xt"] = iopool.tile([P, CW], f32, tag="xt", name="xt")
                    nc.sync.dma_start(out=v["xt"],
                                      in_=x[r0:r0 + P, c0:c0 + CW])
                elif k == 1:
                    v["ps1"] = psum.tile([P, CW], f32, tag="ps", name="ps")
                    nc.tensor.matmul(v["ps1"], lhsT=m1t,
                                     rhs=_bf16_view(v.pop("xt")),
                                     start=True, stop=True)
                elif k == 2:
                    v["y1t"] = midpool.tile([P, CW], f32, tag="y1t",
                                            name="y1t")
                    nc.vector.transpose(out=v["y1t"], in_=v.pop("ps1"))
                elif k == 3:
                    v["ps2"] = psum.tile([P, CW], f32, tag="ps", name="ps")
                    nc.tensor.matmul(v["ps2"], lhsT=m2t,
                                     rhs=_bf16_view(v.pop("y1t")),
                                     start=True, stop=True)
                elif k == 4:
                    v["qi"] = midpool.tile([P, CW], i32, tag="qi", name="qi")
                    nc.scalar.copy(v["qi"], v.pop("ps2"))
                elif k == 5:
                    v["q2"] = midpool.tile([P, CW], bf16, tag="q2", name="q2")
                    nc.vector.tensor_copy(out=v["q2"], in_=v.pop("qi"))
                elif k == 6:
                    v["ps3"] = psum.tile([P, CW], f32, tag="ps", name="ps")
                    nc.tensor.matmul(v["ps3"], lhsT=m3t, rhs=v.pop("q2"),
                                     start=True, stop=True)
                elif k == 7:
                    v["zt"] = midpool.tile([P, CW], f32, tag="zt", name="zt")
                    nc.vector.transpose(out=v["zt"], in_=v.pop("ps3"))
                elif k == 8:
                    v["ps4"] = psum.tile([P, CW], f32, tag="ps", name="ps")
                    nc.tensor.matmul(v["ps4"], lhsT=m4t,
                                     rhs=_bf16_view(v.pop("zt")),
                                     start=True, stop=True)
                elif k == 9:
                    v["o"] = iopool.tile([P, CW], f32, tag="o", name="o")
                    nc.scalar.copy(v["o"], v.pop("ps4"))
                elif k == 10:
                    nc.sync.dma_start(out=y[r0:r0 + P, c0:c0 + CW],
                                      in_=v.pop("o"))

            n_stages = 11
            for t in range(len(chunks) + n_stages - 1):
                for k in range(n_stages - 1, -1, -1):  # deepest stage first
                    i = t - k
                    if 0 <= i < len(chunks):
                        stage(k, i)

    nc.compile()
    _BUILD_CACHE[key] = nc
    return nc


def kernel(x: np.ndarray, block_size, qp, _trace: bool = False,
           _results_out: list | None = None) -> np.ndarray:
    n = int(block_size)
    qp = int(qp)
    b, ch, h, w = x.shape
    assert P % n == 0, f"block size {n} must divide {P}"
    # the 32x32 block-transpose keeps w mod 32 in the partition dim; the row
    # DCT stays block-diagonal iff n divides 32
    assert 32 % n == 0, f"block size {n} must divide 32"
    assert h % n == 0 and w % n == 0, "padding path not implemented"
    assert b == N_CORES, f"expected batch {N_CORES}, got {b}"
    rows = ch * h
    assert rows % P == 0 and w % CW == 0

    qstep = float(np.float32(2.0 ** ((qp - 4.0) / 6.0)))
    d = _dct_matrix(n)
    dbig = np.kron(np.eye(P // n), d)
    m1 = dbig.T            # colDCT:      out = Dbig @ X
    m2 = dbig.T / qstep    # rowDCT/q     (same Dbig in the flipped layout)
    m3 = qstep * dbig      # rowIDCT*q
    m4 = dbig              # colIDCT
    consts = {
        f"m{i}": np.ascontiguousarray(m.astype(ml_dtypes.bfloat16))
        for i, m in enumerate((m1, m2, m3, m4), start=1)
    }

    nc = _build(rows, w)
    x_np = np.asarray(x, dtype=np.float32)
    in_maps = [
        {"x": np.ascontiguousarray(x_np[i].reshape(rows, w)), **consts}
        for i in range(N_CORES)
    ]
    res = bass_utils.run_bass_kernel_spmd(
        nc, in_maps, core_ids=list(range(N_CORES)), trace=_trace)
    if _results_out is not None:
        _results_out.append(res)
    out = np.stack([res.results[i]["y"].reshape(ch, h, w)
                    for i in range(N_CORES)])
    return out


# revision 18
# speedup vs baseline: 1.0191x; 1.0040x over previous
"""Block-DCT quantizer (8x8 DCT -> quant/dequant -> IDCT) on 8 Trainium2 cores.

Sharding: pure data parallel over batch. Core b processes x[b] = [3, 1024, 1024],
flattened to [3072, 1024], in 48 chunks of [128, 512].

Key trick: bf16 is the top half of fp32, so a stride-2 bf16 access pattern over
an fp32 SBUF tile IS its bf16 truncation — and the PE streams it at full bf16
rate (measured). Intermediates therefore stay fp32, each layout flip is a DVE
32x32-block transpose reading PSUM directly (fused with the evacuation), and
matmuls consume bf16 views for free:

    S1  colDCT      ps1 = Dbig @ bf16view(X)         (PE)
    F1  transpose   y1t = blkT(ps1)     PSUM->SBUF   (DVE, fp32)
        partition becomes (h' div 32, w mod 32); w mod 8 stays 8-aligned, so
        the row DCT in this layout is the same block-diagonal Dbig.
    S2  rowDCT/q    ps2 = (Dbig/qstep) @ bf16view(y1t)
    Q1  round       qi  = int32(ps2)    PSUM->SBUF   (ACT, exact RNE cast)
    Q2  cast        q2  = bf16(qi)      SBUF->SBUF   (DVE 2x mode, exact)
    S3  rowIDCT*q   ps3 = (qstep*Dbig^T) @ q2
    F2  transpose   zt  = blkT(ps3)     PSUM->SBUF   (DVE, involution)
    S4  colIDCT     ps4 = Dbig^T @ bf16view(zt)
    E4  evac        o   = ps4           PSUM->SBUF   (ACT)

Quantized coefficients land on exact integers (zero for sane inputs), so bf16
truncation cannot perturb the rounding decision and the output matches the
fp32 reference exactly.

The loop is emitted software-pipelined (one sub-stage per tick, deepest stage
first) so each engine's in-order queue interleaves chunks instead of executing
the serial per-chunk dependency chain.
"""
import math
import sys

sys.path.insert(0, "/opt/trn_rl_repo")

import ml_dtypes
import numpy as np

import concourse.bass as bass  # noqa: F401
import concourse.mybir as mybir
import concourse.tile as tile
from concourse import bacc, bass_utils

P = 128
CW = 512         # chunk width = matmul free dim = one PSUM bank of fp32
N_CORES = 8

_BUILD_CACHE = {}


def _dct_matrix(n: int) -> np.ndarray:
    k = np.arange(n, dtype=np.float64)[:, None]
    j = np.arange(n, dtype=np.float64)[None, :]
    d = np.cos(math.pi / n * (j + 0.5) * k)
    scale = np.full((n, 1), math.sqrt(2.0 / n))
    scale[0, 0] = math.sqrt(1.0 / n)
    return d * scale


def _bf16_view(ap):
    # top 16 bits of each little-endian fp32 element = its bf16 truncation
    return ap.bitcast(mybir.dt.bfloat16)[:, 1::2]


def _build(rows: int, width: int):
    key = (rows, width)
    if key in _BUILD_CACHE:
        return _BUILD_CACHE[key]

    assert rows % P == 0 and width % CW == 0
    n_strips = rows // P
    n_wchunks = width // CW
    f32 = mybir.dt.float32
    bf16 = mybir.dt.bfloat16
    i32 = mybir.dt.int32

    nc = bacc.Bacc("TRN2", target_bir_lowering=False, debug=False,
                   num_devices=N_CORES)
    x = nc.dram_tensor("x", [rows, width], f32, kind="ExternalInput").ap()
    mall = nc.dram_tensor("mall", [P, 4 * P], bf16, kind="ExternalInput").ap()
    y = nc.dram_tensor("y", [rows, width], f32, kind="ExternalOutput").ap()

    with tile.TileContext(nc) as tc:
        with tc.tile_pool(name="consts", bufs=1) as cpool, \
             tc.tile_pool(name="io", bufs=8) as iopool, \
             tc.tile_pool(name="mid", bufs=8) as midpool, \
             tc.tile_pool(name="psum", bufs=8, space="PSUM") as psum:
            mtile = cpool.tile([P, 4 * P], bf16, tag="mall", name="mtile")
            nc.sync.dma_start(out=mtile, in_=mall)
            m1t, m2t, m3t, m4t = (mtile[:, i * P:(i + 1) * P] for i in range(4))

            chunks = [(s * P, c * CW)
                      for s in range(n_strips) for c in range(n_wchunks)]
            st = [dict() for _ in chunks]

            def stage(k, i):
                v = st[i]
                r0, c0 = chunks[i]
                if k == 0:
                    v["xt"] = iopool.tile([P, CW], f32, tag="xt", name="xt")
                    nc.sync.dma_start(out=v["xt"],
                                      in_=x[r0:r0 + P, c0:c0 + CW])
                elif k == 1:
                    v["ps1"] = psum.tile([P, CW], f32, tag="ps", name="ps")
                    nc.tensor.matmul(v["ps1"], lhsT=m1t,
                                     rhs=_bf16_view(v.pop("xt")),
                                     start=True, stop=True)
                elif k == 2:
                    v["y1t"] = midpool.tile([P, CW], f32, tag="y1t",
                                            name="y1t")
                    nc.vector.transpose(out=v["y1t"], in_=v.pop("ps1"))
                elif k == 3:
                    v["ps2"] = psum.tile([P, CW], f32, tag="ps", name="ps")
                    nc.tensor.matmul(v["ps2"], lhsT=m2t,
                                     rhs=_bf16_view(v.pop("y1t")),
                                     start=True, stop=True)
                elif k == 4:
                    v["qi"] = midpool.tile([P, CW], i32, tag="qi", name="qi")
                    nc.scalar.copy(v["qi"], v.pop("ps2"))
                elif k == 5:
                    v["q2"] = midpool.tile([P, CW], bf16, tag="q2", name="q2")
                    nc.vector.tensor_copy(out=v["q2"], in_=v.pop("qi"))
                elif k == 6:
                    v["ps3"] = psum.tile([P, CW], f32, tag="ps", name="ps")
                    nc.tensor.matmul(v["ps3"], lhsT=m3t, rhs=v.pop("q2"),
                                     start=True, stop=True)
                elif k == 7:
                    v["zt"] = midpool.tile([P, CW], f32, tag="zt", name="zt")
                    nc.vector.transpose(out=v["zt"], in_=v.pop("ps3"))
                elif k == 8:
                    v["ps4"] = psum.tile([P, CW], f32, tag="ps", name="ps")
                    nc.tensor.matmul(v["ps4"], lhsT=m4t,
                                     rhs=_bf16_view(v.pop("zt")),
                                     start=True, stop=True)
                elif k == 9:
                    v["o"] = iopool.tile([P, CW], f32, tag="o", name="o")
                    nc.scalar.copy(v["o"], v.pop("ps4"))
                elif k == 10:
                    nc.sync.dma_start(out=y[r0:r0 + P, c0:c0 + CW],
                                      in_=v.pop("o"))

            n_stages = 11
            for t in range(len(chunks) + n_stages - 1):
                for k in range(n_stages - 1, -1, -1):  # deepest stage first
                    i = t - k
                    if 0 <= i < len(chunks):
                        stage(k, i)

    nc.compile()
    _BUILD_CACHE[key] = nc
    return nc


def kernel(x: np.ndarray, block_size, qp, _trace: bool = False,
           _results_out: list | None = None) -> np.ndarray:
    n = int(block_size)
    qp = int(qp)
    b, ch, h, w = x.shape
    assert P % n == 0, f"block size {n} must divide {P}"
    # the 32x32 block-transpose keeps w mod 32 in the partition dim; the row
    # DCT stays block-diagonal iff n divides 32
    assert 32 % n == 0, f"block size {n} must divide 32"
    assert h % n == 0 and w % n == 0, "padding path not implemented"
    assert b == N_CORES, f"expected batch {N_CORES}, got {b}"
    rows = ch * h
    assert rows % P == 0 and w % CW == 0

    qstep = float(np.float32(2.0 ** ((qp - 4.0) / 6.0)))
    d = _dct_matrix(n)
    dbig = np.kron(np.eye(P // n), d)
    m1 = dbig.T            # colDCT:      out = Dbig @ X
    m2 = dbig.T / qstep    # rowDCT/q     (same Dbig in the flipped layout)
    m3 = qstep * dbig      # rowIDCT*q
    m4 = dbig              # colIDCT
    consts = {"mall": np.ascontiguousarray(
        np.concatenate([m1, m2, m3, m4], axis=1).astype(ml_dtypes.bfloat16))}

    nc = _build(rows, w)
    x_np = np.asarray(x, dtype=np.float32)
    in_maps = [
        {"x": np.ascontiguousarray(x_np[i].reshape(rows, w)), **consts}
        for i in range(N_CORES)
    ]
    res = bass_utils.run_bass_kernel_spmd(
        nc, in_maps, core_ids=list(range(N_CORES)), trace=_trace)
    if _results_out is not None:
        _results_out.append(res)
    out = np.stack([res.results[i]["y"].reshape(ch, h, w)
                    for i in range(N_CORES)])
    return out


# revision 19
# speedup vs baseline: 1.0237x; 1.0046x over previous
"""Block-DCT quantizer (8x8 DCT -> quant/dequant -> IDCT) on 8 Trainium2 cores.

Sharding: pure data parallel over batch. Core b processes x[b] = [3, 1024, 1024],
flattened to [3072, 1024], in 48 chunks of [128, 512].

Key trick: bf16 is the top half of fp32, so a stride-2 bf16 access pattern over
an fp32 SBUF tile IS its bf16 truncation — and the PE streams it at full bf16
rate (measured). Intermediates therefore stay fp32, each layout flip is a DVE
32x32-block transpose reading PSUM directly (fused with the evacuation), and
matmuls consume bf16 views for free:

    S1  colDCT      ps1 = Dbig @ bf16view(X)         (PE)
    F1  transpose   y1t = blkT(ps1)     PSUM->SBUF   (DVE, fp32)
        partition becomes (h' div 32, w mod 32); w mod 8 stays 8-aligned, so
        the row DCT in this layout is the same block-diagonal Dbig.
    S2  rowDCT/q    ps2 = (Dbig/qstep) @ bf16view(y1t)
    Q1  round       qi  = int32(ps2)    PSUM->SBUF   (ACT, exact RNE cast)
    Q2  cast        q2  = bf16(qi)      SBUF->SBUF   (DVE 2x mode, exact)
    S3  rowIDCT*q   ps3 = (qstep*Dbig^T) @ q2
    F2  transpose   zt  = blkT(ps3)     PSUM->SBUF   (DVE, involution)
    S4  colIDCT     ps4 = Dbig^T @ bf16view(zt)
    E4  evac        o   = ps4           PSUM->SBUF   (ACT)

Quantized coefficients land on exact integers (zero for sane inputs), so bf16
truncation cannot perturb the rounding decision and the output matches the
fp32 reference exactly.

The loop is emitted software-pipelined (one sub-stage per tick, deepest stage
first) so each engine's in-order queue interleaves chunks instead of executing
the serial per-chunk dependency chain.
"""
import math
import sys

sys.path.insert(0, "/opt/trn_rl_repo")

import ml_dtypes
import numpy as np

import concourse.bass as bass  # noqa: F401
import concourse.mybir as mybir
import concourse.tile as tile
from concourse import bacc, bass_utils

P = 128
CW = 512         # chunk width = matmul free dim = one PSUM bank of fp32
N_CORES = 8

_BUILD_CACHE = {}


def _dct_matrix(n: int) -> np.ndarray:
    k = np.arange(n, dtype=np.float64)[:, None]
    j = np.arange(n, dtype=np.float64)[None, :]
    d = np.cos(math.pi / n * (j + 0.5) * k)
    scale = np.full((n, 1), math.sqrt(2.0 / n))
    scale[0, 0] = math.sqrt(1.0 / n)
    return d * scale


def _bf16_view(ap):
    # top 16 bits of each little-endian fp32 element = its bf16 truncation
    return ap.bitcast(mybir.dt.bfloat16)[:, 1::2]


def _build(rows: int, width: int):
    key = (rows, width)
    if key in _BUILD_CACHE:
        return _BUILD_CACHE[key]

    assert rows % P == 0 and width % CW == 0
    n_strips = rows // P
    n_wchunks = width // CW
    f32 = mybir.dt.float32
    bf16 = mybir.dt.bfloat16
    i32 = mybir.dt.int32

    nc = bacc.Bacc("TRN2", target_bir_lowering=False, debug=False,
                   num_devices=N_CORES)
    x = nc.dram_tensor("x", [rows, width], f32, kind="ExternalInput").ap()
    mall = nc.dram_tensor("mall", [P, 4 * P], bf16, kind="ExternalInput").ap()
    y = nc.dram_tensor("y", [rows, width], f32, kind="ExternalOutput").ap()

    with tile.TileContext(nc) as tc:
        with tc.tile_pool(name="consts", bufs=1) as cpool, \
             tc.tile_pool(name="io", bufs=12) as iopool, \
             tc.tile_pool(name="mid", bufs=12) as midpool, \
             tc.tile_pool(name="psum", bufs=8, space="PSUM") as psum:
            mtile = cpool.tile([P, 4 * P], bf16, tag="mall", name="mtile")
            nc.sync.dma_start(out=mtile, in_=mall)
            m1t, m2t, m3t, m4t = (mtile[:, i * P:(i + 1) * P] for i in range(4))

            chunks = [(s * P, c * CW)
                      for s in range(n_strips) for c in range(n_wchunks)]
            st = [dict() for _ in chunks]

            def stage(k, i):
                v = st[i]
                r0, c0 = chunks[i]
                if k == 0:
                    v["xt"] = iopool.tile([P, CW], f32, tag="xt", name="xt")
                    nc.sync.dma_start(out=v["xt"],
                                      in_=x[r0:r0 + P, c0:c0 + CW])
                elif k == 1:
                    v["ps1"] = psum.tile([P, CW], f32, tag="ps", name="ps")
                    nc.tensor.matmul(v["ps1"], lhsT=m1t,
                                     rhs=_bf16_view(v.pop("xt")),
                                     start=True, stop=True)
                elif k == 2:
                    v["y1t"] = midpool.tile([P, CW], f32, tag="y1t",
                                            name="y1t")
                    nc.vector.transpose(out=v["y1t"], in_=v.pop("ps1"))
                elif k == 3:
                    v["ps2"] = psum.tile([P, CW], f32, tag="ps", name="ps")
                    nc.tensor.matmul(v["ps2"], lhsT=m2t,
                                     rhs=_bf16_view(v.pop("y1t")),
                                     start=True, stop=True)
                elif k == 4:
                    v["qi"] = midpool.tile([P, CW], i32, tag="qi", name="qi")
                    nc.scalar.copy(v["qi"], v.pop("ps2"))
                elif k == 5:
                    v["q2"] = midpool.tile([P, CW], bf16, tag="q2", name="q2")
                    nc.vector.tensor_copy(out=v["q2"], in_=v.pop("qi"))
                elif k == 6:
                    v["ps3"] = psum.tile([P, CW], f32, tag="ps", name="ps")
                    nc.tensor.matmul(v["ps3"], lhsT=m3t, rhs=v.pop("q2"),
                                     start=True, stop=True)
                elif k == 7:
                    v["zt"] = midpool.tile([P, CW], f32, tag="zt", name="zt")
                    nc.vector.transpose(out=v["zt"], in_=v.pop("ps3"))
                elif k == 8:
                    v["ps4"] = psum.tile([P, CW], f32, tag="ps", name="ps")
                    nc.tensor.matmul(v["ps4"], lhsT=m4t,
                                     rhs=_bf16_view(v.pop("zt")),
                                     start=True, stop=True)
                elif k == 9:
                    v["o"] = iopool.tile([P, CW], f32, tag="o", name="o")
                    nc.scalar.copy(v["o"], v.pop("ps4"))
                elif k == 10:
                    nc.sync.dma_start(out=y[r0:r0 + P, c0:c0 + CW],
                                      in_=v.pop("o"))

            n_stages = 11
            for t in range(len(chunks) + n_stages - 1):
                for k in range(n_stages - 1, -1, -1):  # deepest stage first
                    i = t - k
                    if 0 <= i < len(chunks):
                        stage(k, i)

    nc.compile()
    _BUILD_CACHE[key] = nc
    return nc


def kernel(x: np.ndarray, block_size, qp, _trace: bool = False,
           _results_out: list | None = None) -> np.ndarray:
    n = int(block_size)
    qp = int(qp)
    b, ch, h, w = x.shape
    assert P % n == 0, f"block size {n} must divide {P}"
    # the 32x32 block-transpose keeps w mod 32 in the partition dim; the row
    # DCT stays block-diagonal iff n divides 32
    assert 32 % n == 0, f"block size {n} must divide 32"
    assert h % n == 0 and w % n == 0, "padding path not implemented"
    assert b == N_CORES, f"expected batch {N_CORES}, got {b}"
    rows = ch * h
    assert rows % P == 0 and w % CW == 0

    qstep = float(np.float32(2.0 ** ((qp - 4.0) / 6.0)))
    d = _dct_matrix(n)
    dbig = np.kron(np.eye(P // n), d)
    m1 = dbig.T            # colDCT:      out = Dbig @ X
    m2 = dbig.T / qstep    # rowDCT/q     (same Dbig in the flipped layout)
    m3 = qstep * dbig      # rowIDCT*q
    m4 = dbig              # colIDCT
    consts = {"mall": np.ascontiguousarray(
        np.concatenate([m1, m2, m3, m4], axis=1).astype(ml_dtypes.bfloat16))}

    nc = _build(rows, w)
    x_np = np.asarray(x, dtype=np.float32)
    in_maps = [
        {"x": np.ascontiguousarray(x_np[i].reshape(rows, w)), **consts}
        for i in range(N_CORES)
    ]
    res = bass_utils.run_bass_kernel_spmd(
        nc, in_maps, core_ids=list(range(N_CORES)), trace=_trace)
    if _results_out is not None:
        _results_out.append(res)
    out = np.stack([res.results[i]["y"].reshape(ch, h, w)
                    for i in range(N_CORES)])
    return out
